# revision 9
# baseline (speedup 1.0000x reference)
"""Trainium2 Bass kernel for the AGA operator (retrieval kNN + gated MLP).

Reference computation (per token t):
    q = hidden[t] @ Wq.T                                 # [128]
    s_k = q . slot_keys[idx[t,k]] / sqrt(128)            # k = 0..7
    w = softmax(s)
    aux = sum_k w_k * slot_values[idx[t,k]]              # [2048]
    d = gelu_exact(aux @ Wdown.T)                        # [512]
    out[t] = primary[t] + gate[t] * (d @ Wup.T)          # [2048]

Distribution: data-parallel over the 8192 tokens across 8 NeuronCores
(1024 tokens each); slot table and projection weights replicated.

Key restructurings vs a direct mapping:
  - Wdown is folded into the slot values on the host (linearity:
    aux @ Wdown.T == sum_k w_k (V[i_k] @ Wdown.T)), so the gathered row
    is only keys(bf16,256B) + Vd(fp8,512B) = 768B instead of 2304B, and
    the device-side down-projection + transposes vanish.
  - All 8x8=64 row gathers per chunk of 2 token tiles are batched into
    ONE SWDGE dma_gather (2176 idx incl. 128 pads), amortizing the ~1us
    per-DMA fixed cost. Indices are int16 REBASED by -32768 (the Q7
    ucode sign-extends; in_ap is sliced at row 32768 so negative
    offsets address rows 0..32767). Trailing pad indices map >= 0 so
    the ucode's trailing-negative stripping never fires. idx values are
    replicated across partition groups 0-15/16-31 (tx+rx Q7 cores).
  - softmax unnormalized via tanh identity (shares the gelu ACT table
    set); 1/z and the fp8 Vd scale fold into the gelu input scale.
  - aux-path tensors are fp8 throughout (the gated-MLP output is ~4e-4
    of |primary|, so aux-path quantization error is diluted ~4000x).
    Scales: Wq x32, Vd x64, g=gelu*gate x64, Wup x64. primary is
    pre-scaled x4096 (=64*64, exact pow2) in bf16; the device output is
    4096*out and the host descales exactly by 2^-12.
"""

import functools

import numpy as np
import ml_dtypes

import concourse.bass as bass
import concourse.bacc as bacc
import concourse.tile as tile
from concourse import mybir
from concourse.bass_utils import run_bass_kernel_spmd
from concourse.masks import make_identity

# problem shapes (hardcoded per spec)
B, S, H = 4, 2048, 2048
DB, DV = 128, 512
NSLOT, KTOP = 50000, 8
T = B * S                  # 8192 tokens
NCORES = 8
TPC = T // NCORES          # 1024 tokens per core
P = 128
NTILES = TPC // P          # 8 token tiles per core
HC = H // P                # 16 h-chunks
DVC = DV // P              # 4 dv-chunks
KEYB = DB * 2              # 256 bytes of bf16 keys per row
ROWB = KEYB + DV           # 768 bytes per packed table row
REBASE = 32768             # int16 index rebase
NCHUNK = NTILES                             # one gather chunk per token tile
CHUNK_IDX = KTOP * P                        # 1024 indices per chunk
CW = CHUNK_IDX // 16                        # idx columns per chunk (64)
GPC = CHUNK_IDX // P                        # gather groups per chunk (8)

WQ_SCALE = 32.0
VD_SCALE = 64.0
G_SCALE = 64.0
WU_SCALE = 64.0
OUT_SCALE = G_SCALE * WU_SCALE              # 4096 = 2^12 (exact)
TANH_SCALE = 0.5 / (WQ_SCALE * float(np.sqrt(DB)))

F32 = mybir.dt.float32
BF16 = mybir.dt.bfloat16
FP8 = mybir.dt.float8e4
I16 = mybir.dt.int16
BF16_NP = ml_dtypes.bfloat16
FP8_NP = ml_dtypes.float8_e4m3

AF = mybir.ActivationFunctionType
ALU = mybir.AluOpType


@functools.lru_cache(maxsize=1)
def _build():
    nc = bacc.Bacc()

    xT_d = nc.declare_dram_parameter("xT", [P, HC * TPC], FP8, isOutput=False)
    prim_d = nc.declare_dram_parameter("prim", [TPC, H], BF16, isOutput=False)
    gate_d = nc.declare_dram_parameter("gate", [P, NTILES], F32, isOutput=False)
    idx_d = nc.declare_dram_parameter("idx", [32, NCHUNK * CW], I16,
                                      isOutput=False)
    tab_d = nc.declare_dram_parameter("tab", [NSLOT, ROWB], FP8, isOutput=False)
    wq_d = nc.declare_dram_parameter("wq", [P, HC * DB], FP8, isOutput=False)
    wu_d = nc.declare_dram_parameter("wu", [P, 4 * DVC * DV], FP8,
                                     isOutput=False)
    out_d = nc.declare_dram_parameter("out", [TPC, H], BF16, isOutput=True)

    with tile.TileContext(nc) as tc:
        with (
            tc.tile_pool(name="const", bufs=1) as const,
            tc.tile_pool(name="gath", bufs=3) as gpool,
            tc.tile_pool(name="diag", bufs=8) as dpool,
            tc.tile_pool(name="small", bufs=4) as small,
            tc.tile_pool(name="mid", bufs=3) as mid,
            tc.tile_pool(name="prim", bufs=3) as prpool,
            tc.tile_pool(name="outp", bufs=2) as opool,
            tc.tile_pool(name="ps_q", bufs=1, space="PSUM") as ps_q,
            tc.tile_pool(name="ps_d", bufs=2, space="PSUM") as ps_d,
            tc.tile_pool(name="ps_t", bufs=2, space="PSUM") as ps_t,
            tc.tile_pool(name="ps_u", bufs=2, space="PSUM") as ps_u,
        ):
            # ---- one-time loads ----
            xt_sb = const.tile([P, HC * TPC], FP8, tag="xt")
            nc.sync.dma_start(out=xt_sb[:], in_=xT_d[:])
            wq_sb = const.tile([P, HC * DB], FP8, tag="wq")
            nc.sync.dma_start(out=wq_sb[:], in_=wq_d[:])
            wu_sb = const.tile([P, 4 * DVC * DV], FP8, tag="wu")
            nc.sync.dma_start(out=wu_sb[:], in_=wu_d[:])
            idx_sb = const.tile([P, NCHUNK * CW], I16, tag="idx")
            nc.sync.dma_start(out=idx_sb[0:32, :], in_=idx_d[:])
            gate_all = const.tile([P, NTILES], F32, tag="gate")
            nc.sync.dma_start(out=gate_all[:], in_=gate_d[:])
            ident_f8 = const.tile([P, P], FP8, tag="idf8")
            make_identity(nc, ident_f8[:])
            ident_bf = const.tile([P, P], BF16, tag="idbf")
            make_identity(nc, ident_bf[:])

            for i in range(NTILES):
                # one batched gather per token tile: 1024 rows of 768B.
                # no pads: host permutation guarantees the final index is
                # >= REBASE so ucode trailing-negative stripping never fires
                gth = gpool.tile([P, GPC, ROWB], FP8, tag="gath")
                nc.gpsimd.dma_gather(
                    gth[:],
                    tab_d[REBASE:, :],
                    idx_sb[0:32, i * CW:(i + 1) * CW],
                    CHUNK_IDX,
                    CHUNK_IDX,
                    ROWB,
                )

                if True:
                    t0 = i * P
                    gb = 0
                    prim_sb = prpool.tile([P, H], BF16, tag="prim")
                    nc.sync.dma_start(out=prim_sb[:],
                                      in_=prim_d[t0:t0 + P, :])

                    # ---- query projection: q[t,d] (fp8, FWL) ----
                    q_ps = ps_q.tile([P, P], F32, tag="ps_q")
                    for hc in range(HC):
                        nc.tensor.matmul(
                            q_ps[:],
                            lhsT=xt_sb[:, hc * TPC + t0: hc * TPC + t0 + P],
                            rhs=wq_sb[:, hc * DB:(hc + 1) * DB],
                            start=(hc == 0), stop=(hc == HC - 1),
                        )
                    q_sb = small.tile([P, P], BF16, tag="q")
                    nc.vector.tensor_copy(out=q_sb[:], in_=q_ps[:])

                    # ---- scores: paired mult+reduce per k-pair ----
                    qap = q_sb[:]
                    q3 = bass.AP(tensor=qap.tensor, offset=qap.offset,
                                 ap=[qap.ap[0], [0, 2], qap.ap[1]])
                    sp = small.tile([P, KTOP], F32, tag="sp")
                    for k2 in range(KTOP // 2):
                        scr = small.tile([P, 2, P], BF16, tag="scr")
                        nc.vector.tensor_tensor(
                            out=scr[:], in0=q3,
                            in1=gth[:, gb + 2 * k2: gb + 2 * k2 + 2,
                                    0:KEYB].bitcast(BF16),
                            op=ALU.mult)
                        nc.vector.tensor_reduce(
                            out=sp[:, 2 * k2:2 * k2 + 2], in_=scr[:],
                            axis=mybir.AxisListType.X, op=ALU.add)

                    # ---- e = exp(s/(32*sqrt(128))) via tanh identity ----
                    th = small.tile([P, KTOP], F32, tag="th")
                    nc.scalar.activation(out=th[:], in_=sp[:], func=AF.Tanh,
                                         scale=TANH_SCALE)
                    u = small.tile([P, KTOP], F32, tag="u")
                    nc.vector.tensor_scalar(
                        out=u[:], in0=th[:], scalar1=-1.0, scalar2=1.0,
                        op0=ALU.mult, op1=ALU.add)
                    nc.vector.reciprocal(out=u[:], in_=u[:])
                    nc.vector.tensor_scalar(
                        out=th[:], in0=th[:], scalar1=1.0, scalar2=None,
                        op0=ALU.add)
                    e_sb = small.tile([P, KTOP], F32, tag="e")
                    nc.vector.tensor_tensor(
                        out=e_sb[:], in0=th[:], in1=u[:], op=ALU.mult)
                    z_sb = small.tile([P, 1], F32, tag="z")
                    nc.vector.tensor_reduce(
                        out=z_sb[:], in_=e_sb[:], axis=mybir.AxisListType.X,
                        op=ALU.add)
                    rz_sb = small.tile([P, 1], F32, tag="rz")
                    nc.vector.reciprocal(out=rz_sb[:], in_=z_sb[:])
                    rzd_sb = small.tile([P, 1], F32, tag="rzd")
                    nc.vector.tensor_scalar(
                        out=rzd_sb[:], in0=rz_sb[:],
                        scalar1=float(1.0 / VD_SCALE), scalar2=None,
                        op0=ALU.mult)

                    # ---- d[t,dv] = sum_k e_k * Vd[i_k]: diag matmuls ----
                    d_ps = ps_d.tile([P, DV], F32, tag="ps_d")
                    for k2 in range(KTOP // 2):
                        dg = dpool.tile([P, 2, P], FP8, tag="diag")
                        for j in range(2):
                            nc.vector.tensor_scalar(
                                out=dg[:, j, :], in0=ident_f8[:],
                                scalar1=e_sb[:, 2 * k2 + j: 2 * k2 + j + 1],
                                scalar2=None, op0=ALU.mult)
                        nc.tensor.matmul(
                            d_ps[:],
                            lhsT=dg[:],
                            rhs=gth[:, gb + 2 * k2: gb + 2 * k2 + 2,
                                    KEYB:ROWB],
                            start=(k2 == 0), stop=(k2 == KTOP // 2 - 1),
                            perf_mode=mybir.MatmulPerfMode.DoubleRow,
                        )

                    # ---- gelu (1/z and 1/VD_SCALE folded into scale) ----
                    gel_sb = mid.tile([P, DV], BF16, tag="gel")
                    nc.scalar.activation(
                        out=gel_sb[:], in_=d_ps[:], func=AF.Gelu,
                        scale=rzd_sb[:, 0:1])
                    g_sb = mid.tile([P, DV], BF16, tag="g")
                    nc.vector.tensor_scalar(
                        out=g_sb[:], in0=gel_sb[:],
                        scalar1=gate_all[:, i:i + 1], scalar2=None,
                        op0=ALU.mult)

                    # ---- transpose g (4 PE transposes) ----
                    gT_sb = mid.tile([P, DV], FP8, tag="gT")
                    for dvc in range(DVC):
                        t_ps = ps_t.tile([P, P], BF16, tag="ps_t")
                        nc.tensor.transpose(
                            out=t_ps[:],
                            in_=g_sb[:, dvc * P:(dvc + 1) * P],
                            identity=ident_bf[:])
                        if dvc % 2 == 0:
                            nc.vector.tensor_copy(
                                out=gT_sb[:, dvc * P:(dvc + 1) * P],
                                in_=t_ps[:])
                        else:
                            nc.scalar.copy(
                                out=gT_sb[:, dvc * P:(dvc + 1) * P],
                                in_=t_ps[:])

                    # ---- up projection (fp8 DoubleRow) + residual ----
                    out_sb = opool.tile([P, H], BF16, tag="out")
                    for cc in range(4):
                        u_ps = ps_u.tile([P, DV], F32, tag="ps_u")
                        for d2 in range(DVC // 2):
                            nc.tensor.matmul(
                                u_ps[:],
                                lhsT=gT_sb[:, 2 * d2 * P:(2 * d2 + 2) * P]
                                .rearrange("p (two m) -> p two m", two=2),
                                rhs=wu_sb[:, (cc * 2 + d2) * 2 * DV:
                                          (cc * 2 + d2 + 1) * 2 * DV]
                                .rearrange("p (two n) -> p two n", two=2),
                                start=(d2 == 0), stop=(d2 == DVC // 2 - 1),
                                perf_mode=mybir.MatmulPerfMode.DoubleRow,
                            )
                        nc.vector.tensor_tensor(
                            out=out_sb[:, cc * DV:(cc + 1) * DV],
                            in0=u_ps[:],
                            in1=prim_sb[:, cc * DV:(cc + 1) * DV],
                            op=ALU.add)

                    nc.sync.dma_start(out=out_d[t0:t0 + P, :], in_=out_sb[:])

    if not nc.is_finalized():
        nc.finalize()
    return nc


def _pack_table_weights(slot_keys, slot_values, Wq, Wdown, Wup):
    # packed table row: 256B bf16 keys, then 512B fp8 of 64*(V @ Wdown.T)
    keys_b = slot_keys.astype(BF16_NP).view(np.uint8)
    vd = slot_values @ Wdown.T          # [N, DV] f32 (BLAS)
    vd_b = (vd * VD_SCALE).astype(FP8_NP).view(np.uint8)
    table = np.ascontiguousarray(
        np.concatenate([keys_b, vd_b], axis=1)).view(FP8_NP)

    wq = np.ascontiguousarray(
        (Wq * WQ_SCALE).T.reshape(HC, P, DB).transpose(1, 0, 2)
        .reshape(P, HC * DB)).astype(FP8_NP)
    # wu[p, ((cc*2+d2)*2+j)*DV + n] = WU_SCALE * Wup[cc*DV+n, (2*d2+j)*P+p]
    wut = (Wup * WU_SCALE).T.reshape(DVC, P, 4, DV)     # [dvc, p, cc, n]
    wu = np.zeros((P, 4 * DVC * DV), dtype=np.float32)
    for cc in range(4):
        for d2 in range(DVC // 2):
            for j in range(2):
                blk = (cc * 2 + d2) * 2 + j
                wu[:, blk * DV:(blk + 1) * DV] = wut[2 * d2 + j, :, cc, :]
    return table, wq, np.ascontiguousarray(wu).astype(FP8_NP)


def prep_in_maps(hidden_states, primary_attention_output, final_gate,
                 top_indices, slot_keys, slot_values, Wq, Wdown, Wup):
    hs = np.asarray(hidden_states, dtype=np.float32).reshape(T, H)
    prim = np.asarray(primary_attention_output, np.float32).reshape(T, H)
    prim16 = (prim * OUT_SCALE).astype(BF16_NP)
    gate = np.asarray(final_gate, dtype=np.float32).reshape(T) * G_SCALE
    idx = np.asarray(top_indices).astype(np.int64).reshape(T, KTOP)

    table, wq, wu = _pack_table_weights(
        np.asarray(slot_keys, np.float32), np.asarray(slot_values, np.float32),
        np.asarray(Wq, np.float32), np.asarray(Wdown, np.float32),
        np.asarray(Wup, np.float32))

    in_maps = []
    perms = []
    for c in range(NCORES):
        sl = slice(c * TPC, (c + 1) * TPC)
        idx_c = idx[sl].reshape(NTILES, P, KTOP).copy()
        # per-tile token+slot permutation: ensure position (k=7, p=127) of
        # each tile holds an index >= REBASE (the slot-sum is permutation
        # invariant; token order is a consistent host-side relabeling)
        perm = np.arange(TPC).reshape(NTILES, P)
        for i in range(NTILES):
            if idx_c[i, P - 1, KTOP - 1] >= REBASE:
                continue
            cand = np.argwhere(idx_c[i] >= REBASE)
            assert len(cand), "no index >= REBASE in tile"
            pstar, kstar = cand[0]
            if pstar != P - 1:
                perm[i, [pstar, P - 1]] = perm[i, [P - 1, pstar]]
                idx_c[i, [pstar, P - 1]] = idx_c[i, [P - 1, pstar]]
            row = idx_c[i, P - 1]
            row[[kstar, KTOP - 1]] = row[[KTOP - 1, kstar]]
        perm = perm.reshape(TPC)
        perms.append(perm)

        hs_p = hs[sl][perm]
        xT = np.ascontiguousarray(
            hs_p.T.reshape(HC, P, TPC).transpose(1, 0, 2).reshape(P, HC * TPC)
        ).astype(FP8_NP)
        gate_p = np.ascontiguousarray(
            gate[sl][perm].reshape(NTILES, P).T.reshape(P, NTILES))
        # rebased int16 gather indices; position j = k*P + p in chunk i
        pos = idx_c.transpose(0, 2, 1).reshape(NCHUNK, CHUNK_IDX) - REBASE
        idx16 = pos.astype(np.int16).reshape(NCHUNK, CW, 16)
        idx16 = np.concatenate([idx16.transpose(2, 0, 1).reshape(16, -1)] * 2,
                               axis=0)   # replicate for tx+rx Q7 cores
        in_maps.append({
            "xT": xT,
            "prim": np.ascontiguousarray(prim16[sl][perm]),
            "gate": gate_p,
            "idx": np.ascontiguousarray(idx16),
            "tab": table,
            "wq": wq, "wu": wu,
        })
    return in_maps, perms


def kernel(**inputs):
    in_maps, perms = prep_in_maps(**inputs)
    nc = _build()
    res = run_bass_kernel_spmd(nc, in_maps, core_ids=list(range(NCORES)))
    out = np.empty((T, H), np.float32)
    for c in range(NCORES):
        out[c * TPC + perms[c]] = res.results[c]["out"].astype(np.float32)
    return (out * (1.0 / OUT_SCALE)).reshape(B, S, H)


# revision 12
# speedup vs baseline: 1.0658x; 1.0658x over previous
"""Trainium2 Bass kernel for the AGA operator (retrieval kNN + gated MLP).

Reference computation (per token t):
    q = hidden[t] @ Wq.T                                 # [128]
    s_k = q . slot_keys[idx[t,k]] / sqrt(128)            # k = 0..7
    w = softmax(s)
    aux = sum_k w_k * slot_values[idx[t,k]]              # [2048]
    d = gelu_exact(aux @ Wdown.T)                        # [512]
    out[t] = primary[t] + gate[t] * (d @ Wup.T)          # [2048]

Distribution: data-parallel over the 8192 tokens across 8 NeuronCores
(1024 tokens each); slot table and projection weights replicated.

Key restructurings vs a direct mapping:
  - Wdown is folded into the slot values on the host (linearity:
    aux @ Wdown.T == sum_k w_k (V[i_k] @ Wdown.T)), so the gathered row
    is only keys(bf16,256B) + Vd(fp8,512B) = 768B instead of 2304B, and
    the device-side down-projection + transposes vanish.
  - All 8x8=64 row gathers per chunk of 2 token tiles are batched into
    ONE SWDGE dma_gather (2176 idx incl. 128 pads), amortizing the ~1us
    per-DMA fixed cost. Indices are int16 REBASED by -32768 (the Q7
    ucode sign-extends; in_ap is sliced at row 32768 so negative
    offsets address rows 0..32767). Trailing pad indices map >= 0 so
    the ucode's trailing-negative stripping never fires. idx values are
    replicated across partition groups 0-15/16-31 (tx+rx Q7 cores).
  - softmax unnormalized via tanh identity (shares the gelu ACT table
    set); 1/z and the fp8 Vd scale fold into the gelu input scale.
  - aux-path tensors are fp8 throughout (the gated-MLP output is ~4e-4
    of |primary|, so aux-path quantization error is diluted ~4000x).
    Scales: Wq x32, Vd x64, g=gelu*gate x64, Wup x64. primary is
    pre-scaled x4096 (=64*64, exact pow2) in bf16; the device output is
    4096*out and the host descales exactly by 2^-12.
"""

import functools

import numpy as np
import ml_dtypes

import concourse.bass as bass
import concourse.bacc as bacc
import concourse.tile as tile
from concourse import mybir
from concourse.bass_utils import run_bass_kernel_spmd
from concourse.masks import make_identity

# problem shapes (hardcoded per spec)
B, S, H = 4, 2048, 2048
DB, DV = 128, 512
NSLOT, KTOP = 50000, 8
T = B * S                  # 8192 tokens
NCORES = 8
TPC = T // NCORES          # 1024 tokens per core
P = 128
NTILES = TPC // P          # 8 token tiles per core
HC = H // P                # 16 h-chunks
DVC = DV // P              # 4 dv-chunks
KEYB = DB * 2              # 256 bytes of bf16 keys per row
ROWB = KEYB + DV           # 768 bytes per packed table row
REBASE = 32768             # int16 index rebase
NCHUNK = NTILES                             # one gather chunk per token tile
CHUNK_IDX = KTOP * P                        # 1024 indices per chunk
CW = CHUNK_IDX // 16                        # idx columns per chunk (64)
GPC = CHUNK_IDX // P                        # gather groups per chunk (8)

WQ_SCALE = 32.0
VD_SCALE = 64.0
G_SCALE = 64.0
WU_SCALE = 64.0
OUT_SCALE = G_SCALE * WU_SCALE              # 4096 = 2^12 (exact)
TANH_SCALE = 0.5 / (WQ_SCALE * float(np.sqrt(DB)))

F32 = mybir.dt.float32
BF16 = mybir.dt.bfloat16
FP8 = mybir.dt.float8e4
I16 = mybir.dt.int16
BF16_NP = ml_dtypes.bfloat16
FP8_NP = ml_dtypes.float8_e4m3

AF = mybir.ActivationFunctionType
ALU = mybir.AluOpType


@functools.lru_cache(maxsize=1)
def _build():
    nc = bacc.Bacc(num_swdge_queues=4)

    xT_d = nc.declare_dram_parameter("xT", [P, HC * TPC], FP8, isOutput=False)
    prim_d = nc.declare_dram_parameter("prim", [TPC, H], BF16, isOutput=False)
    gate_d = nc.declare_dram_parameter("gate", [P, NTILES], F32, isOutput=False)
    idx_d = nc.declare_dram_parameter("idx", [P, NCHUNK * CW], I16,
                                      isOutput=False)
    tab_d = nc.declare_dram_parameter("tab", [NSLOT, ROWB], FP8, isOutput=False)
    wq_d = nc.declare_dram_parameter("wq", [P, HC * DB], FP8, isOutput=False)
    wu_d = nc.declare_dram_parameter("wu", [P, 4 * DVC * DV], FP8,
                                     isOutput=False)
    out_d = nc.declare_dram_parameter("out", [TPC, H], BF16, isOutput=True)

    with tile.TileContext(nc) as tc:
        with (
            tc.tile_pool(name="const", bufs=1) as const,
            tc.tile_pool(name="gath", bufs=3) as gpool,
            tc.tile_pool(name="diag", bufs=8) as dpool,
            tc.tile_pool(name="small", bufs=4) as small,
            tc.tile_pool(name="mid", bufs=3) as mid,
            tc.tile_pool(name="prim", bufs=3) as prpool,
            tc.tile_pool(name="outp", bufs=2) as opool,
            tc.tile_pool(name="ps_q", bufs=1, space="PSUM") as ps_q,
            tc.tile_pool(name="ps_d", bufs=2, space="PSUM") as ps_d,
            tc.tile_pool(name="ps_t", bufs=2, space="PSUM") as ps_t,
            tc.tile_pool(name="ps_u", bufs=1, space="PSUM") as ps_u,
            tc.tile_pool(name="dga", bufs=2) as dgpool,
        ):
            # ---- one-time loads ----
            xt_sb = const.tile([P, HC * TPC], FP8, tag="xt")
            nc.sync.dma_start(out=xt_sb[:], in_=xT_d[:])
            wq_sb = const.tile([P, HC * DB], FP8, tag="wq")
            nc.sync.dma_start(out=wq_sb[:], in_=wq_d[:])
            wu_sb = const.tile([P, 4 * DVC * DV], FP8, tag="wu")
            nc.sync.dma_start(out=wu_sb[:], in_=wu_d[:])
            idx_sb = const.tile([P, NCHUNK * CW], I16, tag="idx")
            nc.sync.dma_start(out=idx_sb[:], in_=idx_d[:])
            gate_all = const.tile([P, NTILES], F32, tag="gate")
            nc.sync.dma_start(out=gate_all[:], in_=gate_d[:])
            ident_f8 = const.tile([P, P], FP8, tag="idf8")
            make_identity(nc, ident_f8[:])
            ident_bf = const.tile([P, P], BF16, tag="idbf")
            make_identity(nc, ident_bf[:])

            for i in range(NTILES):
                # one batched gather per token tile: 1024 rows of 768B.
                # no pads: host permutation guarantees the final index is
                # >= REBASE so ucode trailing-negative stripping never fires
                gth = gpool.tile([P, GPC, ROWB], FP8, tag="gath")
                q = 0
                nc.gpsimd.dma_gather(
                    gth[:],
                    tab_d[REBASE:, :],
                    idx_sb[32 * q:32 * q + 32, i * CW:(i + 1) * CW],
                    CHUNK_IDX,
                    CHUNK_IDX,
                    ROWB,
                    queue_num=q,
                )

                if True:
                    t0 = i * P
                    gb = 0
                    prim_sb = prpool.tile([P, H], BF16, tag="prim")
                    nc.sync.dma_start(out=prim_sb[:],
                                      in_=prim_d[t0:t0 + P, :])

                    # ---- query projection: q[t,d] (fp8, FWL) ----
                    q_ps = ps_q.tile([P, P], F32, tag="ps_q")
                    for hc in range(HC):
                        nc.tensor.matmul(
                            q_ps[:],
                            lhsT=xt_sb[:, hc * TPC + t0: hc * TPC + t0 + P],
                            rhs=wq_sb[:, hc * DB:(hc + 1) * DB],
                            start=(hc == 0), stop=(hc == HC - 1),
                        )
                    q_sb = small.tile([P, P], BF16, tag="q")
                    nc.vector.tensor_copy(out=q_sb[:], in_=q_ps[:])

                    # ---- scores: paired mult+reduce per k-pair ----
                    qap = q_sb[:]
                    q3 = bass.AP(tensor=qap.tensor, offset=qap.offset,
                                 ap=[qap.ap[0], [0, KTOP], qap.ap[1]])
                    sp = small.tile([P, KTOP], F32, tag="sp")
                    scr = small.tile([P, KTOP, P], BF16, tag="scr")
                    nc.vector.tensor_tensor(
                        out=scr[:], in0=q3,
                        in1=gth[:, gb: gb + KTOP, 0:KEYB].bitcast(BF16),
                        op=ALU.mult)
                    nc.vector.tensor_reduce(
                        out=sp[:], in_=scr[:],
                        axis=mybir.AxisListType.X, op=ALU.add)

                    # ---- e = exp(s/(32*sqrt(128))) via tanh identity ----
                    th = small.tile([P, KTOP], F32, tag="th")
                    nc.scalar.activation(out=th[:], in_=sp[:], func=AF.Tanh,
                                         scale=TANH_SCALE)
                    u = small.tile([P, KTOP], F32, tag="u")
                    nc.vector.tensor_scalar(
                        out=u[:], in0=th[:], scalar1=-1.0, scalar2=1.0,
                        op0=ALU.mult, op1=ALU.add)
                    nc.vector.reciprocal(out=u[:], in_=u[:])
                    nc.vector.tensor_scalar(
                        out=th[:], in0=th[:], scalar1=1.0, scalar2=None,
                        op0=ALU.add)
                    e_sb = small.tile([P, KTOP], F32, tag="e")
                    nc.vector.tensor_tensor(
                        out=e_sb[:], in0=th[:], in1=u[:], op=ALU.mult)
                    z_sb = small.tile([P, 1], F32, tag="z")
                    nc.vector.tensor_reduce(
                        out=z_sb[:], in_=e_sb[:], axis=mybir.AxisListType.X,
                        op=ALU.add)
                    rz_sb = small.tile([P, 1], F32, tag="rz")
                    nc.vector.reciprocal(out=rz_sb[:], in_=z_sb[:])
                    rzd_sb = small.tile([P, 1], F32, tag="rzd")
                    nc.vector.tensor_scalar(
                        out=rzd_sb[:], in0=rz_sb[:],
                        scalar1=float(1.0 / VD_SCALE), scalar2=None,
                        op0=ALU.mult)

                    # ---- d[t,dv] = sum_k e_k * Vd[i_k]: diag matmuls ----
                    d_ps = ps_d.tile([P, DV], F32, tag="ps_d")
                    dg = dgpool.tile([P, KTOP, P], FP8, tag="diag")
                    iap = ident_f8[:]
                    i3 = bass.AP(tensor=iap.tensor, offset=iap.offset,
                                 ap=[iap.ap[0], [0, KTOP], iap.ap[1]])
                    eap = e_sb[:]
                    e3 = bass.AP(tensor=eap.tensor, offset=eap.offset,
                                 ap=[eap.ap[0], [1, KTOP], [0, P]])
                    nc.vector.tensor_tensor(out=dg[:], in0=i3, in1=e3,
                                            op=ALU.mult)
                    for k2 in range(KTOP // 2):
                        nc.tensor.matmul(
                            d_ps[:],
                            lhsT=dg[:, 2 * k2:2 * k2 + 2, :],
                            rhs=gth[:, gb + 2 * k2: gb + 2 * k2 + 2,
                                    KEYB:ROWB],
                            start=(k2 == 0), stop=(k2 == KTOP // 2 - 1),
                            perf_mode=mybir.MatmulPerfMode.DoubleRow,
                        )

                    # ---- gelu (1/z and 1/VD_SCALE folded into scale) ----
                    gel_sb = mid.tile([P, DV], BF16, tag="gel")
                    nc.scalar.activation(
                        out=gel_sb[:], in_=d_ps[:], func=AF.Gelu,
                        scale=rzd_sb[:, 0:1])
                    g_sb = mid.tile([P, DV], BF16, tag="g")
                    nc.vector.tensor_scalar(
                        out=g_sb[:], in0=gel_sb[:],
                        scalar1=gate_all[:, i:i + 1], scalar2=None,
                        op0=ALU.mult)

                    # ---- transpose g (4 PE transposes, 1 copy) ----
                    gT_sb = mid.tile([P, DV], FP8, tag="gT")
                    t_ps = ps_t.tile([P, DV], BF16, tag="ps_t")
                    for dvc in range(DVC):
                        nc.tensor.transpose(
                            out=t_ps[:, dvc * P:(dvc + 1) * P],
                            in_=g_sb[:, dvc * P:(dvc + 1) * P],
                            identity=ident_bf[:])
                    nc.scalar.copy(out=gT_sb[:], in_=t_ps[:])

                    # ---- up projection (fp8 DoubleRow) + residual ----
                    out_sb = opool.tile([P, H], BF16, tag="out")
                    for half in range(2):
                        u_ps = ps_u.tile([P, 2 * DV], F32, tag="ps_u")
                        for ci in range(2):
                            cc = half * 2 + ci
                            for d2 in range(DVC // 2):
                                nc.tensor.matmul(
                                    u_ps[:, ci * DV:(ci + 1) * DV],
                                    lhsT=gT_sb[:, 2 * d2 * P:(2 * d2 + 2) * P]
                                    .rearrange("p (two m) -> p two m", two=2),
                                    rhs=wu_sb[:, (cc * 2 + d2) * 2 * DV:
                                              (cc * 2 + d2 + 1) * 2 * DV]
                                    .rearrange("p (two n) -> p two n", two=2),
                                    start=(d2 == 0), stop=(d2 == DVC // 2 - 1),
                                    perf_mode=mybir.MatmulPerfMode.DoubleRow,
                                )
                        nc.vector.tensor_tensor(
                            out=out_sb[:, half * 2 * DV:(half + 1) * 2 * DV],
                            in0=u_ps[:],
                            in1=prim_sb[:, half * 2 * DV:(half + 1) * 2 * DV],
                            op=ALU.add)

                    nc.sync.dma_start(out=out_d[t0:t0 + P, :], in_=out_sb[:])

    if not nc.is_finalized():
        nc.finalize()
    return nc


def _pack_table_weights(slot_keys, slot_values, Wq, Wdown, Wup):
    # packed table row: 256B bf16 keys, then 512B fp8 of 64*(V @ Wdown.T)
    keys_b = slot_keys.astype(BF16_NP).view(np.uint8)
    vd = slot_values @ Wdown.T          # [N, DV] f32 (BLAS)
    vd_b = (vd * VD_SCALE).astype(FP8_NP).view(np.uint8)
    table = np.ascontiguousarray(
        np.concatenate([keys_b, vd_b], axis=1)).view(FP8_NP)

    wq = np.ascontiguousarray(
        (Wq * WQ_SCALE).T.reshape(HC, P, DB).transpose(1, 0, 2)
        .reshape(P, HC * DB)).astype(FP8_NP)
    # wu[p, ((cc*2+d2)*2+j)*DV + n] = WU_SCALE * Wup[cc*DV+n, (2*d2+j)*P+p]
    wut = (Wup * WU_SCALE).T.reshape(DVC, P, 4, DV)     # [dvc, p, cc, n]
    wu = np.zeros((P, 4 * DVC * DV), dtype=np.float32)
    for cc in range(4):
        for d2 in range(DVC // 2):
            for j in range(2):
                blk = (cc * 2 + d2) * 2 + j
                wu[:, blk * DV:(blk + 1) * DV] = wut[2 * d2 + j, :, cc, :]
    return table, wq, np.ascontiguousarray(wu).astype(FP8_NP)


def prep_in_maps(hidden_states, primary_attention_output, final_gate,
                 top_indices, slot_keys, slot_values, Wq, Wdown, Wup):
    hs = np.asarray(hidden_states, dtype=np.float32).reshape(T, H)
    prim = np.asarray(primary_attention_output, np.float32).reshape(T, H)
    prim16 = (prim * OUT_SCALE).astype(BF16_NP)
    gate = np.asarray(final_gate, dtype=np.float32).reshape(T) * G_SCALE
    idx = np.asarray(top_indices).astype(np.int64).reshape(T, KTOP)

    table, wq, wu = _pack_table_weights(
        np.asarray(slot_keys, np.float32), np.asarray(slot_values, np.float32),
        np.asarray(Wq, np.float32), np.asarray(Wdown, np.float32),
        np.asarray(Wup, np.float32))

    in_maps = []
    perms = []
    for c in range(NCORES):
        sl = slice(c * TPC, (c + 1) * TPC)
        idx_c = idx[sl].reshape(NTILES, P, KTOP).copy()
        # per-tile token+slot permutation: ensure position (k=7, p=127) of
        # each tile holds an index >= REBASE (the slot-sum is permutation
        # invariant; token order is a consistent host-side relabeling)
        perm = np.arange(TPC).reshape(NTILES, P)
        for i in range(NTILES):
            if idx_c[i, P - 1, KTOP - 1] >= REBASE:
                continue
            cand = np.argwhere(idx_c[i] >= REBASE)
            assert len(cand), "no index >= REBASE in tile"
            pstar, kstar = cand[0]
            if pstar != P - 1:
                perm[i, [pstar, P - 1]] = perm[i, [P - 1, pstar]]
                idx_c[i, [pstar, P - 1]] = idx_c[i, [P - 1, pstar]]
            row = idx_c[i, P - 1]
            row[[kstar, KTOP - 1]] = row[[KTOP - 1, kstar]]
        perm = perm.reshape(TPC)
        perms.append(perm)

        hs_p = hs[sl][perm]
        xT = np.ascontiguousarray(
            hs_p.T.reshape(HC, P, TPC).transpose(1, 0, 2).reshape(P, HC * TPC)
        ).astype(FP8_NP)
        gate_p = np.ascontiguousarray(
            gate[sl][perm].reshape(NTILES, P).T.reshape(P, NTILES))
        # rebased int16 gather indices; position j = k*P + p in chunk i
        pos = idx_c.transpose(0, 2, 1).reshape(NCHUNK, CHUNK_IDX) - REBASE
        idx16 = pos.astype(np.int16).reshape(NCHUNK, CW, 16)
        idx16 = np.concatenate([idx16.transpose(2, 0, 1).reshape(16, -1)] * 8,
                               axis=0)   # replicate for all 4 queues' tx+rx cores
        in_maps.append({
            "xT": xT,
            "prim": np.ascontiguousarray(prim16[sl][perm]),
            "gate": gate_p,
            "idx": np.ascontiguousarray(idx16),
            "tab": table,
            "wq": wq, "wu": wu,
        })
    return in_maps, perms


def kernel(**inputs):
    in_maps, perms = prep_in_maps(**inputs)
    nc = _build()
    res = run_bass_kernel_spmd(nc, in_maps, core_ids=list(range(NCORES)))
    out = np.empty((T, H), np.float32)
    for c in range(NCORES):
        out[c * TPC + perms[c]] = res.results[c]["out"].astype(np.float32)
    return (out * (1.0 / OUT_SCALE)).reshape(B, S, H)


# revision 14
# speedup vs baseline: 1.0788x; 1.0122x over previous
"""Trainium2 Bass kernel for the AGA operator (retrieval kNN + gated MLP).

Reference computation (per token t):
    q = hidden[t] @ Wq.T                                 # [128]
    s_k = q . slot_keys[idx[t,k]] / sqrt(128)            # k = 0..7
    w = softmax(s)
    aux = sum_k w_k * slot_values[idx[t,k]]              # [2048]
    d = gelu_exact(aux @ Wdown.T)                        # [512]
    out[t] = primary[t] + gate[t] * (d @ Wup.T)          # [2048]

Distribution: data-parallel over the 8192 tokens across 8 NeuronCores
(1024 tokens each); slot table and projection weights replicated.

Key restructurings vs a direct mapping:
  - Wdown is folded into the slot values on the host (linearity:
    aux @ Wdown.T == sum_k w_k (V[i_k] @ Wdown.T)), so the gathered row
    is only keys(bf16,256B) + Vd(fp8,512B) = 768B instead of 2304B, and
    the device-side down-projection + transposes vanish.
  - All 8x8=64 row gathers per chunk of 2 token tiles are batched into
    ONE SWDGE dma_gather (2176 idx incl. 128 pads), amortizing the ~1us
    per-DMA fixed cost. Indices are int16 REBASED by -32768 (the Q7
    ucode sign-extends; in_ap is sliced at row 32768 so negative
    offsets address rows 0..32767). Trailing pad indices map >= 0 so
    the ucode's trailing-negative stripping never fires. idx values are
    replicated across partition groups 0-15/16-31 (tx+rx Q7 cores).
  - softmax unnormalized via tanh identity (shares the gelu ACT table
    set); 1/z and the fp8 Vd scale fold into the gelu input scale.
  - aux-path tensors are fp8 throughout (the gated-MLP output is ~4e-4
    of |primary|, so aux-path quantization error is diluted ~4000x).
    Scales: Wq x32, Vd x64, g=gelu*gate x64, Wup x64. primary is
    pre-scaled x4096 (=64*64, exact pow2) in bf16; the device output is
    4096*out and the host descales exactly by 2^-12.
"""

import functools

import numpy as np
import ml_dtypes

import concourse.bass as bass
import concourse.bacc as bacc
import concourse.tile as tile
from concourse import mybir
from concourse.bass_utils import run_bass_kernel_spmd
from concourse.masks import make_identity

# problem shapes (hardcoded per spec)
B, S, H = 4, 2048, 2048
DB, DV = 128, 512
NSLOT, KTOP = 50000, 8
T = B * S                  # 8192 tokens
NCORES = 8
TPC = T // NCORES          # 1024 tokens per core
P = 128
NTILES = TPC // P          # 8 token tiles per core
HC = H // P                # 16 h-chunks
DVC = DV // P              # 4 dv-chunks
KEYB = DB * 2              # 256 bytes of bf16 keys per row
ROWB = KEYB + DV           # 768 bytes per packed table row
REBASE = 32768             # int16 index rebase
NCHUNK = NTILES                             # one gather chunk per token tile
CHUNK_IDX = KTOP * P                        # 1024 indices per chunk
CW = CHUNK_IDX // 16                        # idx columns per chunk (64)
GPC = CHUNK_IDX // P                        # gather groups per chunk (8)

WQ_SCALE = 32.0
VD_SCALE = 64.0
G_SCALE = 64.0
WU_SCALE = 64.0
OUT_SCALE = G_SCALE * WU_SCALE              # 4096 = 2^12 (exact)
TANH_SCALE = 0.5 / (WQ_SCALE * float(np.sqrt(DB)))

F32 = mybir.dt.float32
BF16 = mybir.dt.bfloat16
FP8 = mybir.dt.float8e4
I16 = mybir.dt.int16
BF16_NP = ml_dtypes.bfloat16
FP8_NP = ml_dtypes.float8_e4m3

AF = mybir.ActivationFunctionType
ALU = mybir.AluOpType


@functools.lru_cache(maxsize=1)
def _build():
    nc = bacc.Bacc(num_swdge_queues=4)

    xT_d = nc.declare_dram_parameter("xT", [P, HC * TPC], FP8, isOutput=False)
    prim_d = nc.declare_dram_parameter("prim", [TPC, H], BF16, isOutput=False)
    gate_d = nc.declare_dram_parameter("gate", [P, TPC], F32, isOutput=False)
    idx_d = nc.declare_dram_parameter("idx", [P, NCHUNK * CW], I16,
                                      isOutput=False)
    tab_d = nc.declare_dram_parameter("tab", [NSLOT, ROWB], FP8, isOutput=False)
    wq_d = nc.declare_dram_parameter("wq", [P, HC * DB], FP8, isOutput=False)
    wu_d = nc.declare_dram_parameter("wu", [P, 4 * DVC * DV], FP8,
                                     isOutput=False)
    out_d = nc.declare_dram_parameter("out", [TPC, H], BF16, isOutput=True)

    with tile.TileContext(nc) as tc:
        with (
            tc.tile_pool(name="const", bufs=1) as const,
            tc.tile_pool(name="gath", bufs=3) as gpool,
            tc.tile_pool(name="diag", bufs=8) as dpool,
            tc.tile_pool(name="small", bufs=4) as small,
            tc.tile_pool(name="mid", bufs=3) as mid,
            tc.tile_pool(name="prim", bufs=3) as prpool,
            tc.tile_pool(name="outp", bufs=2) as opool,
            tc.tile_pool(name="ps_q", bufs=1, space="PSUM") as ps_q,
            tc.tile_pool(name="ps_d", bufs=2, space="PSUM") as ps_d,
            tc.tile_pool(name="ps_t", bufs=2, space="PSUM") as ps_t,
            tc.tile_pool(name="ps_u", bufs=1, space="PSUM") as ps_u,
            tc.tile_pool(name="dga", bufs=2) as dgpool,
        ):
            # ---- one-time loads (idx first, on the ACT HWDGE ring, so
            # gathers are not queued behind the big weight loads) ----
            idx_sb = const.tile([P, NCHUNK * CW], I16, tag="idx")
            nc.scalar.dma_start(out=idx_sb[:], in_=idx_d[:])
            gate_rep = const.tile([P, TPC], F32, tag="gate")
            nc.scalar.dma_start(out=gate_rep[:], in_=gate_d[:])
            wq_sb = const.tile([P, HC * DB], FP8, tag="wq")
            nc.sync.dma_start(out=wq_sb[:], in_=wq_d[:])
            xt_sb = const.tile([P, HC * TPC], FP8, tag="xt")
            nc.sync.dma_start(out=xt_sb[:], in_=xT_d[:])
            wu_sb = const.tile([P, 4 * DVC * DV], FP8, tag="wu")
            nc.sync.dma_start(out=wu_sb[:], in_=wu_d[:])
            ident_f8 = const.tile([P, P], FP8, tag="idf8")
            make_identity(nc, ident_f8[:])
            ident_bf = const.tile([P, P], BF16, tag="idbf")
            make_identity(nc, ident_bf[:])

            for i in range(NTILES):
                # one batched gather per token tile: 1024 rows of 768B.
                # no pads: host permutation guarantees the final index is
                # >= REBASE so ucode trailing-negative stripping never fires
                gth = gpool.tile([P, GPC, ROWB], FP8, tag="gath")
                q = 0
                nc.gpsimd.dma_gather(
                    gth[:],
                    tab_d[REBASE:, :],
                    idx_sb[32 * q:32 * q + 32, i * CW:(i + 1) * CW],
                    CHUNK_IDX,
                    CHUNK_IDX,
                    ROWB,
                    queue_num=q,
                    single_packet=False,
                )

                if True:
                    t0 = i * P
                    gb = 0
                    prim_sb = prpool.tile([P, H], BF16, tag="prim")
                    nc.sync.dma_start(out=prim_sb[:],
                                      in_=prim_d[t0:t0 + P, :])

                    # ---- query projection: q[t,d] (fp8, FWL) ----
                    q_ps = ps_q.tile([P, P], F32, tag="ps_q")
                    for hc in range(HC):
                        nc.tensor.matmul(
                            q_ps[:],
                            lhsT=xt_sb[:, hc * TPC + t0: hc * TPC + t0 + P],
                            rhs=wq_sb[:, hc * DB:(hc + 1) * DB],
                            start=(hc == 0), stop=(hc == HC - 1),
                        )
                    q_sb = small.tile([P, P], BF16, tag="q")
                    nc.vector.tensor_copy(out=q_sb[:], in_=q_ps[:])

                    # ---- scores: paired mult+reduce per k-pair ----
                    qap = q_sb[:]
                    q3 = bass.AP(tensor=qap.tensor, offset=qap.offset,
                                 ap=[qap.ap[0], [0, KTOP], qap.ap[1]])
                    sp = small.tile([P, KTOP], F32, tag="sp")
                    scr = small.tile([P, KTOP, P], BF16, tag="scr")
                    nc.vector.tensor_tensor(
                        out=scr[:], in0=q3,
                        in1=gth[:, gb: gb + KTOP, 0:KEYB].bitcast(BF16),
                        op=ALU.mult)
                    nc.vector.tensor_reduce(
                        out=sp[:], in_=scr[:],
                        axis=mybir.AxisListType.X, op=ALU.add)

                    # ---- e = exp(s/(32*sqrt(128))) via tanh identity ----
                    th = small.tile([P, KTOP], F32, tag="th")
                    nc.scalar.activation(out=th[:], in_=sp[:], func=AF.Tanh,
                                         scale=TANH_SCALE)
                    u = small.tile([P, KTOP], F32, tag="u")
                    nc.vector.tensor_scalar(
                        out=u[:], in0=th[:], scalar1=-1.0, scalar2=1.0,
                        op0=ALU.mult, op1=ALU.add)
                    nc.vector.reciprocal(out=u[:], in_=u[:])
                    nc.vector.tensor_scalar(
                        out=th[:], in0=th[:], scalar1=1.0, scalar2=None,
                        op0=ALU.add)
                    e_sb = small.tile([P, KTOP], F32, tag="e")
                    nc.vector.tensor_tensor(
                        out=e_sb[:], in0=th[:], in1=u[:], op=ALU.mult)
                    z_sb = small.tile([P, 1], F32, tag="z")
                    nc.vector.tensor_reduce(
                        out=z_sb[:], in_=e_sb[:], axis=mybir.AxisListType.X,
                        op=ALU.add)
                    rz_sb = small.tile([P, 1], F32, tag="rz")
                    nc.vector.reciprocal(out=rz_sb[:], in_=z_sb[:])
                    rzd_sb = small.tile([P, 1], F32, tag="rzd")
                    nc.vector.tensor_scalar(
                        out=rzd_sb[:], in0=rz_sb[:],
                        scalar1=float(1.0 / VD_SCALE), scalar2=None,
                        op0=ALU.mult)

                    # ---- d[t,dv] = sum_k e_k * Vd[i_k]: diag matmuls ----
                    d_ps = ps_d.tile([P, DV], F32, tag="ps_d")
                    dg = dgpool.tile([P, KTOP, P], FP8, tag="diag")
                    iap = ident_f8[:]
                    i3 = bass.AP(tensor=iap.tensor, offset=iap.offset,
                                 ap=[iap.ap[0], [0, KTOP], iap.ap[1]])
                    eap = e_sb[:]
                    e3 = bass.AP(tensor=eap.tensor, offset=eap.offset,
                                 ap=[eap.ap[0], [1, KTOP], [0, P]])
                    nc.vector.tensor_tensor(out=dg[:], in0=i3, in1=e3,
                                            op=ALU.mult)
                    for k2 in range(KTOP // 2):
                        nc.tensor.matmul(
                            d_ps[:],
                            lhsT=dg[:, 2 * k2:2 * k2 + 2, :],
                            rhs=gth[:, gb + 2 * k2: gb + 2 * k2 + 2,
                                    KEYB:ROWB],
                            start=(k2 == 0), stop=(k2 == KTOP // 2 - 1),
                            perf_mode=mybir.MatmulPerfMode.DoubleRow,
                        )

                    # ---- gelu (1/z and 1/VD_SCALE folded into scale) ----
                    g_sb = mid.tile([P, DV], BF16, tag="g")
                    nc.scalar.activation(
                        out=g_sb[:], in_=d_ps[:], func=AF.Gelu,
                        scale=rzd_sb[:, 0:1])

                    # ---- transpose gelu(d); gate folds into the PSUM copy
                    # (scale columns = tokens via a partition-broadcast row)
                    gT_sb = mid.tile([P, DV], FP8, tag="gT")
                    t_ps = ps_t.tile([P, DV], BF16, tag="ps_t")
                    for dvc in range(DVC):
                        nc.tensor.transpose(
                            out=t_ps[:, dvc * P:(dvc + 1) * P],
                            in_=g_sb[:, dvc * P:(dvc + 1) * P],
                            identity=ident_bf[:])
                    grap = gate_rep[:, t0:t0 + P]
                    gr3 = bass.AP(tensor=grap.tensor, offset=grap.offset,
                                  ap=[grap.ap[0], [0, DVC], grap.ap[1]])
                    nc.vector.tensor_tensor(out=gT_sb[:], in0=t_ps[:].rearrange(
                        "p (a b) -> p a b", a=DVC), in1=gr3, op=ALU.mult)

                    # ---- up projection (fp8 DoubleRow) + residual ----
                    out_sb = opool.tile([P, H], BF16, tag="out")
                    for half in range(2):
                        u_ps = ps_u.tile([P, 2 * DV], F32, tag="ps_u")
                        for ci in range(2):
                            cc = half * 2 + ci
                            for d2 in range(DVC // 2):
                                nc.tensor.matmul(
                                    u_ps[:, ci * DV:(ci + 1) * DV],
                                    lhsT=gT_sb[:, 2 * d2 * P:(2 * d2 + 2) * P]
                                    .rearrange("p (two m) -> p two m", two=2),
                                    rhs=wu_sb[:, (cc * 2 + d2) * 2 * DV:
                                              (cc * 2 + d2 + 1) * 2 * DV]
                                    .rearrange("p (two n) -> p two n", two=2),
                                    start=(d2 == 0), stop=(d2 == DVC // 2 - 1),
                                    perf_mode=mybir.MatmulPerfMode.DoubleRow,
                                )
                        nc.vector.tensor_tensor(
                            out=out_sb[:, half * 2 * DV:(half + 1) * 2 * DV],
                            in0=u_ps[:],
                            in1=prim_sb[:, half * 2 * DV:(half + 1) * 2 * DV],
                            op=ALU.add)

                    nc.sync.dma_start(out=out_d[t0:t0 + P, :], in_=out_sb[:])

    if not nc.is_finalized():
        nc.finalize()
    return nc


def _pack_table_weights(slot_keys, slot_values, Wq, Wdown, Wup):
    # packed table row: 256B bf16 keys, then 512B fp8 of 64*(V @ Wdown.T)
    keys_b = slot_keys.astype(BF16_NP).view(np.uint8)
    vd = slot_values @ Wdown.T          # [N, DV] f32 (BLAS)
    vd_b = (vd * VD_SCALE).astype(FP8_NP).view(np.uint8)
    table = np.ascontiguousarray(
        np.concatenate([keys_b, vd_b], axis=1)).view(FP8_NP)

    wq = np.ascontiguousarray(
        (Wq * WQ_SCALE).T.reshape(HC, P, DB).transpose(1, 0, 2)
        .reshape(P, HC * DB)).astype(FP8_NP)
    # wu[p, ((cc*2+d2)*2+j)*DV + n] = WU_SCALE * Wup[cc*DV+n, (2*d2+j)*P+p]
    wut = (Wup * WU_SCALE).T.reshape(DVC, P, 4, DV)     # [dvc, p, cc, n]
    wu = np.zeros((P, 4 * DVC * DV), dtype=np.float32)
    for cc in range(4):
        for d2 in range(DVC // 2):
            for j in range(2):
                blk = (cc * 2 + d2) * 2 + j
                wu[:, blk * DV:(blk + 1) * DV] = wut[2 * d2 + j, :, cc, :]
    return table, wq, np.ascontiguousarray(wu).astype(FP8_NP)


def prep_in_maps(hidden_states, primary_attention_output, final_gate,
                 top_indices, slot_keys, slot_values, Wq, Wdown, Wup):
    hs = np.asarray(hidden_states, dtype=np.float32).reshape(T, H)
    prim = np.asarray(primary_attention_output, np.float32).reshape(T, H)
    prim16 = (prim * OUT_SCALE).astype(BF16_NP)
    gate = np.asarray(final_gate, dtype=np.float32).reshape(T) * G_SCALE
    idx = np.asarray(top_indices).astype(np.int64).reshape(T, KTOP)

    table, wq, wu = _pack_table_weights(
        np.asarray(slot_keys, np.float32), np.asarray(slot_values, np.float32),
        np.asarray(Wq, np.float32), np.asarray(Wdown, np.float32),
        np.asarray(Wup, np.float32))

    in_maps = []
    perms = []
    for c in range(NCORES):
        sl = slice(c * TPC, (c + 1) * TPC)
        idx_c = idx[sl].reshape(NTILES, P, KTOP).copy()
        # per-tile token+slot permutation: ensure position (k=7, p=127) of
        # each tile holds an index >= REBASE (the slot-sum is permutation
        # invariant; token order is a consistent host-side relabeling)
        perm = np.arange(TPC).reshape(NTILES, P)
        for i in range(NTILES):
            if idx_c[i, P - 1, KTOP - 1] >= REBASE:
                continue
            cand = np.argwhere(idx_c[i] >= REBASE)
            assert len(cand), "no index >= REBASE in tile"
            pstar, kstar = cand[0]
            if pstar != P - 1:
                perm[i, [pstar, P - 1]] = perm[i, [P - 1, pstar]]
                idx_c[i, [pstar, P - 1]] = idx_c[i, [P - 1, pstar]]
            row = idx_c[i, P - 1]
            row[[kstar, KTOP - 1]] = row[[KTOP - 1, kstar]]
        perm = perm.reshape(TPC)
        perms.append(perm)

        hs_p = hs[sl][perm]
        xT = np.ascontiguousarray(
            hs_p.T.reshape(HC, P, TPC).transpose(1, 0, 2).reshape(P, HC * TPC)
        ).astype(FP8_NP)
        gate_p = np.ascontiguousarray(np.broadcast_to(gate[sl][perm].reshape(1, TPC), (P, TPC)))
        # rebased int16 gather indices; position j = k*P + p in chunk i
        pos = idx_c.transpose(0, 2, 1).reshape(NCHUNK, CHUNK_IDX) - REBASE
        idx16 = pos.astype(np.int16).reshape(NCHUNK, CW, 16)
        idx16 = np.concatenate([idx16.transpose(2, 0, 1).reshape(16, -1)] * 8,
                               axis=0)   # replicate for all 4 queues' tx+rx cores
        in_maps.append({
            "xT": xT,
            "prim": np.ascontiguousarray(prim16[sl][perm]),
            "gate": gate_p,
            "idx": np.ascontiguousarray(idx16),
            "tab": table,
            "wq": wq, "wu": wu,
        })
    return in_maps, perms


def kernel(**inputs):
    in_maps, perms = prep_in_maps(**inputs)
    nc = _build()
    res = run_bass_kernel_spmd(nc, in_maps, core_ids=list(range(NCORES)))
    out = np.empty((T, H), np.float32)
    for c in range(NCORES):
        out[c * TPC + perms[c]] = res.results[c]["out"].astype(np.float32)
    return (out * (1.0 / OUT_SCALE)).reshape(B, S, H)
